# revision 25
# baseline (speedup 1.0000x reference)
# Multi-head attention (B=2, S=2048, E=1024, H=16) on 8 TRN2 NeuronCores.
#
# Sharding: data-parallel over the 2 batches x tensor-parallel over 4 head
# groups (4 heads each).  Core c handles batch c//4, heads 4*(c%4)..4*(c%4)+3.
# Each core computes its heads' Q/K/V projections, attention, and a partial
# o_proj over its value features; the host sums the 4 partials per batch.
#
# Device-side layout (same host contract as the earlier version):
#  - All matmul inputs are consumed in transposed form (contraction dim on
#    partitions); the host pre-transposes x and the weight shards.
#  - Masked keys are compacted away on the host: only kept tokens (plus zero
#    padding up to KT_LEN) participate in K/V.  Padding slots get an additive
#    -30000 bias so exp() underflows to exactly 0.
#  - Logits are built transposed ([k, q]); the softmax denominator falls out
#    of the AV matmul via an extra all-ones column appended to V.
#
# Schedule (the actual rewrite vs the first version):
#  - The Activation engine's exp() stream is the attention-phase floor
#    (~72 x [128,1024] exp instrs).  Everything else is arranged so the
#    Tensor engine never starves and all PSUM banks stay hot:
#  - Attention runs q-PAIR-major: each (head, key-tile) step computes logits
#    for 1024 queries in one 2-bank PSUM tile -> ONE exp instr (per-instr
#    ACT overhead is ~230ns, so big exps matter).
#  - Q projection for q-chunks 2,3 is interleaved as Tensor-engine filler
#    into pair 0's attention; o_proj for pair-0 tokens is interleaved into
#    pair 1's attention.  Only pair 1's o_proj (8 token tiles) remains as a
#    tail, with stores streaming throughout.
#  - PSUM budget: lg pool 2x[128,1024] (4 banks) + vals/sumexp pool
#    3x[65,512] (3 banks) + filler pool 1x[128,512] (1 bank) = 8 banks.
#  - PSUM evictions go to DVE (projections, normalize) and Pool/GpSimd
#    (o_proj staging), keeping the Activation engine exclusively on exp.
#  - DMA is issued in consumption order with few, fat transfers.

import numpy as np

KT_LEN = 1152  # default compacted+padded key extent (9 tiles of 128)
B, S, E = 2, 2048, 1024
HEADS_PER_CORE = 4
D = 64
N_CORES = 8
ET = E // 128  # 8 contraction tiles for projections
QTILES = S // 512  # 4 query chunks of 512
TT = S // 128  # 16 token tiles of 128

_compiled_nc = {}


def _build_bass(kt_len=KT_LEN):
    import concourse.mybir as mybir
    import concourse.tile as tile
    from concourse import bacc

    f32 = mybir.dt.float32
    f32r = mybir.dt.float32r
    bf16 = mybir.dt.bfloat16
    Exp = mybir.ActivationFunctionType.Exp
    KT_LEN = kt_len
    KT = KT_LEN // 128
    HPC = HEADS_PER_CORE

    nc = bacc.Bacc(None, target_bir_lowering=False, debug=False)

    xT_d = nc.dram_tensor("xT", [E, S], bf16, kind="ExternalInput")
    xkvT_d = nc.dram_tensor("xkvT", [E, KT_LEN], bf16, kind="ExternalInput")
    wqT_d = nc.dram_tensor("wqT", [E, 256], bf16, kind="ExternalInput")
    wkT_d = nc.dram_tensor("wkT", [E, 256], bf16, kind="ExternalInput")
    wvT_d = nc.dram_tensor("wvT", [E, 256], bf16, kind="ExternalInput")
    woT_d = nc.dram_tensor("woT", [256, E], f32r, kind="ExternalInput")
    mb_d = nc.dram_tensor("mbias", [KT_LEN], f32, kind="ExternalInput")
    out_d = nc.dram_tensor("out", [S, E], bf16, kind="ExternalOutput")

    xT_v = xT_d[:].rearrange("(a p) t -> p a t", p=128)  # [128, 8, 2048]
    xkvT_v = xkvT_d[:].rearrange("(a p) t -> p a t", p=128)  # [128, 8, KT_LEN]
    wqT_v = wqT_d[:].rearrange("(a p) d -> p a d", p=128)  # [128, 8, 256]
    wkT_v = wkT_d[:].rearrange("(a p) d -> p a d", p=128)
    wvT_v = wvT_d[:].rearrange("(a p) d -> p a d", p=128)
    woT_v = woT_d[:].rearrange("(a p) e -> p a e", p=128)  # [128, 2, 1024]
    mb_v = mb_d[:].rearrange("(k p) -> p k", p=128)  # [128, KT]

    with tile.TileContext(nc) as tc:
        with (
            tc.tile_pool(name="singles", bufs=1) as singles,
            tc.tile_pool(name="xstream", bufs=3) as xstream,
            tc.tile_pool(name="expool", bufs=4) as expool,
            tc.tile_pool(name="small", bufs=3) as small,
            tc.tile_pool(name="outst", bufs=4) as outst,
            # PSUM: 8 banks total, statically reserved:
            #   lgp  = 2 x [128,1024] (2 banks each) -> 4 banks
            #   valsp= 3 x [65,512]   (1 bank each)  -> 3 banks
            #   opp  = 1 x [128,512]  (1 bank)       -> 1 bank
            tc.tile_pool(name="lgp", bufs=2, space="PSUM") as lgp,
            tc.tile_pool(name="valsp", bufs=3, space="PSUM") as valsp,
            tc.tile_pool(name="opp", bufs=1, space="PSUM") as opp,
        ):
            wq_sb = singles.tile([128, ET, 256], bf16)
            wk_sb = singles.tile([128, ET, 256], bf16)
            wv_sb = singles.tile([128, ET, 256], bf16)
            wo_sb = singles.tile([128, 2, E], f32r)
            # xkv split into two et-halves (separate tensors so the DMA
            # dependency of each half is exact): K accumulates its first
            # half while the second is still in flight.
            KSPL = ET // 2
            xkv_a = singles.tile([128, KSPL, KT_LEN], bf16)
            xkv_b = singles.tile([128, ET - KSPL, KT_LEN], bf16)
            mb_sb = singles.tile([128, KT], f32)
            qT_sb = singles.tile([128, 2, S // 2], bf16)
            # q-chunks 2/3 land in their own tensor (written by the filler
            # projections inside pair-0 attention) so pair-1 reads never
            # serialize against unrelated writes.
            qT2_sb = singles.tile([128, 2, S // 2], bf16)
            kT_sb = singles.tile([128, HPC, KT_LEN], bf16)
            v1_sb = singles.tile([128, KT, HPC, 65], f32r)
            valsTa = singles.tile([128, S], f32r)
            valsTb = singles.tile([128, S], f32r)

            def xkv_et(et):
                return xkv_a[:, et] if et < KSPL else xkv_b[:, et - KSPL]

            # ---- DMA prologue, in consumption order (sync queue = FIFO).
            nc.sync.dma_start(wq_sb, wqT_v)
            nc.sync.dma_start(mb_sb, mb_v)
            xq = {}
            # qc0 arrives in two et-halves so its projection starts sooner.
            xq0a = xstream.tile([128, ET // 2, 512], bf16, tag="x0", name="xq0a")
            nc.sync.dma_start(xq0a, xT_v[:, 0 : ET // 2, 0:512])
            xq0b = xstream.tile([128, ET // 2, 512], bf16, tag="x0", name="xq0b")
            nc.sync.dma_start(xq0b, xT_v[:, ET // 2 :, 0:512])
            nc.sync.dma_start(wk_sb, wkT_v)
            nc.sync.dma_start(xkv_a, xkvT_v[:, 0:KSPL])
            nc.sync.dma_start(wv_sb, wvT_v)
            nc.sync.dma_start(xkv_b, xkvT_v[:, KSPL:])
            xq[1] = xstream.tile([128, ET, 512], bf16, tag="xs", name="xq1")
            nc.sync.dma_start(xq[1], xT_v[:, :, 512:1024])
            # qc2/qc3 reuse the xstream bufs; their dma_starts wait on the
            # Q0/Q1 projections, by which time everything above has issued.
            for qc in (2, 3):
                xq[qc] = xstream.tile(
                    [128, ET, 512], bf16, tag="xs", name=f"xq{qc}"
                )
                nc.sync.dma_start(xq[qc], xT_v[:, :, qc * 512 : (qc + 1) * 512])
            nc.sync.dma_start(wo_sb, woT_v)

            # ---- constants / zero-fill (off the critical DMA+PE path)
            ones_sb = singles.tile([128, 1], f32)
            nc.vector.memset(ones_sb, 1.0)
            ones64 = singles.tile([65, 64], f32r)
            nc.scalar.copy(
                ones64[64:65, :], ones_sb[64:65, 0:1].to_broadcast([1, 64])
            )
            # unused partition halves of kT must be 0 (each head only fills
            # 64 of the 128 contraction partitions)
            zeros_sb = singles.tile([128, 1], f32)
            nc.vector.memset(zeros_sb, 0.0)
            nc.scalar.copy(
                kT_sb, zeros_sb.to_broadcast([128, HPC, KT_LEN])
            )
            nc.scalar.copy(
                v1_sb[:, :, :, 64:65],
                ones_sb.to_broadcast([128, KT, HPC, 1]),
            )

            # ---- Q projection for one 512-query chunk (full-rate version,
            # used for qc0/qc1 before attention starts).
            def q_proj_full(qc):
                psq = lgp.tile([128, 1024], f32, tag="lg", name=f"psq_{qc}")
                for et in range(ET):
                    if qc == 0:
                        rhs = (xq0a if et < ET // 2 else xq0b)[:, et % (ET // 2)]
                    else:
                        rhs = xq[qc][:, et]
                    for bl in range(2):
                        nc.tensor.matmul(
                            psq[:, bl * 512 : (bl + 1) * 512],
                            lhsT=wq_sb[:, et, bl * 128 : (bl + 1) * 128],
                            rhs=rhs,
                            start=(et == 0),
                            stop=(et == ET - 1),
                        )
                for bl in range(2):
                    nc.vector.tensor_copy(
                        qT_sb[:, bl, qc * 512 : (qc + 1) * 512],
                        psq[:, bl * 512 : (bl + 1) * 512],
                    )

            # ---- K^T projection layout: [256 d, KT_LEN] in 3 chunks x 2
            # blocks.  With <=7 groups they all stay open across the PSUM
            # banks so the accumulation runs in two et-stages: stage 0
            # (first xkv half) is sandwiched between the Q0 and Q1
            # projections while the second half is still streaming in.
            nch = (KT_LEN + 511) // 512
            base = KT_LEN // nch // 128 * 128
            KCH = []
            t0 = 0
            for ci in range(nch):
                tw = KT_LEN - t0 if ci == nch - 1 else base
                KCH.append((t0, tw))
                t0 += tw
            groups = [(bl, t0, tw) for bl in range(2) for t0, tw in KCH]

            def k_stage(homes, stage):
                ets = range(0, KSPL) if stage == 0 else range(KSPL, ET)
                for gi, (bl, t0, tw) in enumerate(groups):
                    for et in ets:
                        nc.tensor.matmul(
                            homes[gi],
                            lhsT=wk_sb[:, et, bl * 128 : (bl + 1) * 128],
                            rhs=xkv_et(et)[:, t0 : t0 + tw],
                            start=(et == 0),
                            stop=(et == ET - 1),
                        )

            q_proj_full(0)
            if len(groups) <= 7:
                pskL = lgp.tile([128, 1024], f32, tag="lg", name="pskL")
                homes = []
                for gi, (bl, t0, tw) in enumerate(groups):
                    if gi == 0:
                        homes.append(pskL[:, 0:tw])
                    elif gi == 1:
                        homes.append(pskL[:, 512 : 512 + tw])
                    elif gi < 5:
                        homes.append(
                            valsp.tile([128, tw], f32, tag="vals", name=f"pskv_{gi}")
                        )
                    else:
                        homes.append(
                            opp.tile([128, tw], f32, tag="op", name=f"psko_{gi}")
                        )
                k_stage(homes, 0)
                k_stage(homes, 1)
                for gi, (bl, t0, tw) in enumerate(groups):
                    nc.vector.tensor_copy(
                        kT_sb[0:64, 2 * bl, t0 : t0 + tw], homes[gi][0:64, :]
                    )
                    nc.vector.tensor_copy(
                        kT_sb[64:128, 2 * bl + 1, t0 : t0 + tw], homes[gi][64:128, :]
                    )
            else:
                for bl in range(2):
                    for t0, tw in KCH:
                        psk = lgp.tile([128, 1024], f32, tag="lg", name=f"psk_{bl}_{t0}")
                        for et in range(ET):
                            nc.tensor.matmul(
                                psk[:, :tw],
                                lhsT=wk_sb[:, et, bl * 128 : (bl + 1) * 128],
                                rhs=xkv_et(et)[:, t0 : t0 + tw],
                                start=(et == 0),
                                stop=(et == ET - 1),
                            )
                        nc.vector.tensor_copy(
                            kT_sb[0:64, 2 * bl, t0 : t0 + tw], psk[0:64, :tw]
                        )
                        nc.vector.tensor_copy(
                            kT_sb[64:128, 2 * bl + 1, t0 : t0 + tw], psk[64:128, :tw]
                        )

            # ---- V projection: per token-tile [128 t, 256 d]
            for vt in range(KT):
                psv = lgp.tile([128, 1024], f32, tag="lg", name=f"psv_{vt}")
                for et in range(ET):
                    nc.tensor.matmul(
                        psv[:, :256],
                        lhsT=xkv_et(et)[:, vt * 128 : (vt + 1) * 128],
                        rhs=wv_sb[:, et],
                        start=(et == 0),
                        stop=(et == ET - 1),
                    )
                nc.vector.tensor_copy(
                    v1_sb[:, vt, :, 0:64],
                    psv[:, :256].rearrange("p (h d) -> p h d", h=HPC),
                )

            # Q1 last: its x chunk is the final prologue DMA, so the V
            # projection runs during that transfer instead of idling.
            q_proj_full(1)

            # ---- PE filler generators -------------------------------------
            # Pair 0 fillers: Q projection for qc2/qc3, one (qc, bl) chunk
            # split across two filler slots (4 et-steps each) in the opp bank.
            qfill_state = {}

            def emit_q_filler(i):
                qc = 2 + i // 4
                bl = (i // 2) % 2
                half = i % 2
                if half == 0:
                    qfill_state["t"] = opp.tile(
                        [128, 512], f32, tag="op", name=f"psq2_{qc}_{bl}"
                    )
                t = qfill_state["t"]
                for e4 in range(4):
                    et = half * 4 + e4
                    nc.tensor.matmul(
                        t,
                        lhsT=wq_sb[:, et, bl * 128 : (bl + 1) * 128],
                        rhs=xq[qc][:, et],
                        start=(et == 0),
                        stop=(et == ET - 1),
                    )
                if half == 1:
                    nc.vector.tensor_copy(
                        qT2_sb[:, bl, (qc - 2) * 512 : (qc - 1) * 512], t
                    )

            # Pair 1 fillers: o_proj half-tiles for pair-0 tokens (tt 0..7).
            def emit_op_filler(j):
                ttn, nt = j // 2, j % 2
                op = opp.tile([128, 512], f32, tag="op", name=f"op_{ttn}_{nt}")
                nc.tensor.matmul(
                    op,
                    lhsT=valsTa[:, ttn * 128 : (ttn + 1) * 128],
                    rhs=wo_sb[:, 0, nt * 512 : (nt + 1) * 512],
                    start=True,
                    stop=False,
                )
                nc.tensor.matmul(
                    op,
                    lhsT=valsTb[:, ttn * 128 : (ttn + 1) * 128],
                    rhs=wo_sb[:, 1, nt * 512 : (nt + 1) * 512],
                    start=False,
                    stop=True,
                )
                ot = outst.tile([128, 1024], bf16, tag="ot", name=f"ot_{ttn}_{nt}")
                nc.vector.tensor_copy(ot[:, 0:512], op)
                nc.sync.dma_start(
                    out_d[ttn * 128 : (ttn + 1) * 128, nt * 512 : (nt + 1) * 512],
                    ot[:, 0:512],
                )

            # ---- softmax-normalize one head's accumulated values ----------
            # The sumexp row (partition 64 of the AV accumulator) is spread
            # across 64 partitions on the idle GpSimd engine, keeping the
            # Tensor engine free of broadcast matmuls and the vals PSUM pool
            # free of sumexp scratch tiles.
            def emit_norm(p, h, valsA, valsB, fast=False):
                bl = h // 2
                off = (h % 2) * 64
                vT = valsTa if bl == 0 else valsTb
                uvs = []
                for X, vals in (("A", valsA), ("B", valsB)):
                    uv = small.tile([65, 512], f32r, tag="uv", name=f"uv_{p}_{h}_{X}")
                    nc.vector.tensor_copy(uv, vals)
                    uvs.append(uv)
                for xi, uv in enumerate(uvs):
                    qoff = p * 1024 + xi * 512
                    # broadcast sumexp across 64 partitions with a K=1 matmul
                    # (212ns on PE; every off-PE broadcast path measured
                    # slower end-to-end).  The tail-critical final head uses
                    # the freed filler bank instead of the vals rotation.
                    pool, tag = (opp, "op") if fast else (valsp, "vals")
                    se = pool.tile([64, 512], f32, tag=tag, name=f"se_{p}_{h}_{xi}")
                    nc.tensor.matmul(
                        se,
                        lhsT=ones64[64:65, :],
                        rhs=uv[64:65, :],
                        start=True,
                        stop=True,
                    )
                    rb = small.tile([64, 512], f32, tag="rb", name=f"rb_{p}_{h}_{xi}")
                    nc.vector.reciprocal_approx_fast(rb, se)
                    if off != 0:
                        nc.vector.tensor_mul(
                            vT[0:64, qoff : qoff + 512], uv[0:64, :], rb
                        )
                    else:
                        vn = small.tile(
                            [64, 512], f32r, tag="vn", bufs=2, name=f"vn_{p}_{h}_{xi}"
                        )
                        nc.vector.tensor_mul(vn, uv[0:64, :], rb)
                        nc.gpsimd.dma_start(vT[64:128, qoff : qoff + 512], vn)

            # ---- attention, q-pair-major ----------------------------------
            pending_norm = None
            for p in range(2):
                q0 = p * 1024
                if p == 0:
                    fillers = [(s, emit_q_filler) for s in range(8)]
                    fill_slots = {3, 7}
                else:
                    fillers = [(s, emit_op_filler) for s in range(16)]
                    fill_slots = {3, 5, 7, 8}
                fi = 0
                for h in range(HPC):
                    bl = h // 2
                    valsA = valsB = None
                    for kt in range(KT):
                        lg = lgp.tile([128, 1024], f32, tag="lg", name=f"lg_{p}_{h}_{kt}")
                        qsrc = qT_sb if p == 0 else qT2_sb
                        for xi in range(2):
                            nc.tensor.matmul(
                                lg[:, xi * 512 : (xi + 1) * 512],
                                lhsT=kT_sb[:, h, kt * 128 : (kt + 1) * 128],
                                rhs=qsrc[:, bl, xi * 512 : (xi + 1) * 512],
                                start=True,
                                stop=True,
                            )
                        ex = expool.tile([128, 1024], f32r, tag="ex", name=f"ex_{p}_{h}_{kt}")
                        nc.scalar.activation(
                            ex, lg, Exp, bias=mb_sb[:, kt : kt + 1], scale=0.125
                        )
                        # previous head's normalize lands between this head's
                        # first QKT and first AV in Tensor-engine order, so
                        # its tiny broadcast matmuls ride the exp wait.  The
                        # vals tiles are allocated after it so the pool
                        # rotation (bufs=3) frees banks in dependency order.
                        if kt == 0:
                            if pending_norm is not None:
                                emit_norm(*pending_norm)
                                pending_norm = None
                            valsA = valsp.tile(
                                [65, 512], f32, tag="vals", name=f"vals_{p}_{h}_A"
                            )
                            valsB = valsp.tile(
                                [65, 512], f32, tag="vals", name=f"vals_{p}_{h}_B"
                            )
                        nc.tensor.matmul(
                            valsA,
                            lhsT=v1_sb[:, kt, h],
                            rhs=ex[:, 0:512],
                            start=(kt == 0),
                            stop=(kt == KT - 1),
                        )
                        nc.tensor.matmul(
                            valsB,
                            lhsT=v1_sb[:, kt, h],
                            rhs=ex[:, 512:1024],
                            start=(kt == 0),
                            stop=(kt == KT - 1),
                        )
                        if fi < len(fillers) and (h * KT + kt) % KT in fill_slots:
                            fillers[fi][1](fillers[fi][0])
                            fi += 1
                    pending_norm = (p, h, valsA, valsB)
                # drain any leftover fillers (KT-size changes etc.)
                while fi < len(fillers):
                    fillers[fi][1](fillers[fi][0])
                    fi += 1

            emit_norm(*pending_norm, fast=True)

            # ---- o_proj for pair-1 tokens (tt 8..15): nothing left to
            # overlap with, so spread 16 half-tiles over all 8 freed PSUM
            # banks and drain with both the Scalar and Vector engines.
            def op_homes():
                while True:
                    lga = lgp.tile([128, 1024], f32, tag="lg", name="opfA")
                    lgb = lgp.tile([128, 1024], f32, tag="lg", name="opfB")
                    yield lga[:, 0:512]
                    yield lga[:, 512:1024]
                    yield lgb[:, 0:512]
                    yield lgb[:, 512:1024]
                    for k in range(3):
                        yield valsp.tile([128, 512], f32, tag="vals", name=f"opfv{k}")
                    yield opp.tile([128, 512], f32, tag="op", name="opfo")

            homegen = op_homes()
            for ttn in range(TT // 2, TT):
                ot = outst.tile([128, 1024], bf16, tag="ot", name=f"otf_{ttn}")
                for ntn in range(2):
                    op = next(homegen)
                    nc.tensor.matmul(
                        op,
                        lhsT=valsTa[:, ttn * 128 : (ttn + 1) * 128],
                        rhs=wo_sb[:, 0, ntn * 512 : (ntn + 1) * 512],
                        start=True,
                        stop=False,
                    )
                    nc.tensor.matmul(
                        op,
                        lhsT=valsTb[:, ttn * 128 : (ttn + 1) * 128],
                        rhs=wo_sb[:, 1, ntn * 512 : (ntn + 1) * 512],
                        start=False,
                        stop=True,
                    )
                    # halves drain on different engines; one fat store per tile
                    if ntn == 0:
                        nc.scalar.copy(ot[:, 0:512], op)
                    else:
                        nc.vector.tensor_copy(ot[:, 512:1024], op)
                nc.sync.dma_start(out_d[ttn * 128 : (ttn + 1) * 128, :], ot)

    nc.compile()
    return nc


def _get_nc(kt_len=KT_LEN):
    if kt_len not in _compiled_nc:
        _compiled_nc[kt_len] = _build_bass(kt_len)
    return _compiled_nc[kt_len]


def pick_kt_len(src_padding_mask):
    """Smallest supported compacted key extent covering every batch's kept
    tokens (KT_LEN default covers it with ~5 sigma of slack for random
    masks; anything larger falls back to a wider, slower build)."""
    need = int(np.max(np.sum(np.asarray(src_padding_mask), axis=1)))
    need = max(need, 256)
    need = (need + 127) // 128 * 128
    return KT_LEN if need <= KT_LEN else need


def make_in_maps(x, src_padding_mask, w_qkv, w_o, kt_len=None):
    """Shard the full inputs into the 8 per-core input maps."""
    import ml_dtypes

    bf16 = ml_dtypes.bfloat16
    if kt_len is None:
        kt_len = pick_kt_len(src_padding_mask)
    x = np.asarray(x, dtype=np.float32)
    mask = np.asarray(src_padding_mask)
    w_qkv = np.asarray(w_qkv, dtype=np.float32)
    w_o = np.asarray(w_o, dtype=np.float32)

    # w_qkv rows are per-head interleaved: head h -> rows [192h, 192h+192),
    # split 64/64/64 into q/k/v.
    wr = w_qkv.reshape(16, 3, D, E)  # [head, qkv, d, e]

    in_maps = []
    per_batch = {}
    for b in range(B):
        xb = x[b]  # [S, E]
        xT = np.ascontiguousarray(xb.T)
        idx = np.nonzero(mask[b])[0]
        nk = len(idx)
        assert nk <= kt_len, f"kept keys {nk} exceed kt_len {kt_len}"
        xkvT = np.zeros((E, kt_len), np.float32)
        xkvT[:, :nk] = xb[idx].T
        mb = np.full((kt_len,), -30000.0, np.float32)
        mb[:nk] = 0.0
        per_batch[b] = (xT, xkvT, mb)

    for c in range(N_CORES):
        b, g = divmod(c, N_CORES // B)
        xT, xkvT, mb = per_batch[b]
        heads = slice(g * HEADS_PER_CORE, (g + 1) * HEADS_PER_CORE)
        wq = wr[heads, 0].reshape(256, E)  # [4*64, E]
        wk = wr[heads, 1].reshape(256, E)
        wv = wr[heads, 2].reshape(256, E)
        in_maps.append(
            {
                "xT": xT.astype(bf16),
                "xkvT": xkvT.astype(bf16),
                "wqT": np.ascontiguousarray(wq.T).astype(bf16),
                "wkT": np.ascontiguousarray(wk.T).astype(bf16),
                "wvT": np.ascontiguousarray(wv.T).astype(bf16),
                "woT": np.ascontiguousarray(
                    w_o[:, g * 256 : (g + 1) * 256]
                    .reshape(E, 2, 2, D)[:, :, ::-1, :]
                    .reshape(E, 256)
                    .T
                ),
                "mbias": mb,
            }
        )
    return in_maps


def combine_outputs(outs):
    """Sum the 4 per-head-group partials for each batch."""
    full = np.zeros((B, S, E), np.float32)
    for c in range(N_CORES):
        full[c // (N_CORES // B)] += np.asarray(outs[c]).astype(np.float32)
    return full


def kernel(x, src_padding_mask, w_qkv, w_o, _trace=False):
    from concourse.bass_utils import run_bass_kernel_spmd

    kt_len = pick_kt_len(src_padding_mask)
    nc = _get_nc(kt_len)
    in_maps = make_in_maps(x, src_padding_mask, w_qkv, w_o, kt_len)
    kwargs = {}
    if _trace:
        kwargs = dict(trace=True, trace_cores=list(range(N_CORES)))
    res = run_bass_kernel_spmd(nc, in_maps, core_ids=list(range(N_CORES)), **kwargs)
    out = combine_outputs([r["out"] for r in res.results])
    if _trace:
        kernel._last_result = res
    return out


# revision 26
# speedup vs baseline: 1.0023x; 1.0023x over previous
# Multi-head attention (B=2, S=2048, E=1024, H=16) on 8 TRN2 NeuronCores.
#
# Sharding: data-parallel over the 2 batches x tensor-parallel over 4 head
# groups (4 heads each).  Core c handles batch c//4, heads 4*(c%4)..4*(c%4)+3.
# Each core computes its heads' Q/K/V projections, attention, and a partial
# o_proj over its value features; the host sums the 4 partials per batch.
#
# Device-side layout (same host contract as the earlier version):
#  - All matmul inputs are consumed in transposed form (contraction dim on
#    partitions); the host pre-transposes x and the weight shards.
#  - Masked keys are compacted away on the host: only kept tokens (plus zero
#    padding up to KT_LEN) participate in K/V.  Padding slots get an additive
#    -30000 bias so exp() underflows to exactly 0.
#  - Logits are built transposed ([k, q]); the softmax denominator falls out
#    of the AV matmul via an extra all-ones column appended to V.
#
# Schedule (the actual rewrite vs the first version):
#  - The Activation engine's exp() stream is the attention-phase floor
#    (~72 x [128,1024] exp instrs).  Everything else is arranged so the
#    Tensor engine never starves and all PSUM banks stay hot:
#  - Attention runs q-PAIR-major: each (head, key-tile) step computes logits
#    for 1024 queries in one 2-bank PSUM tile -> ONE exp instr (per-instr
#    ACT overhead is ~230ns, so big exps matter).
#  - Q projection for q-chunks 2,3 is interleaved as Tensor-engine filler
#    into pair 0's attention; o_proj for pair-0 tokens is interleaved into
#    pair 1's attention.  Only pair 1's o_proj (8 token tiles) remains as a
#    tail, with stores streaming throughout.
#  - PSUM budget: lg pool 2x[128,1024] (4 banks) + vals/sumexp pool
#    3x[65,512] (3 banks) + filler pool 1x[128,512] (1 bank) = 8 banks.
#  - PSUM evictions go to DVE (projections, normalize) and Pool/GpSimd
#    (o_proj staging), keeping the Activation engine exclusively on exp.
#  - DMA is issued in consumption order with few, fat transfers.

import numpy as np

KT_LEN = 1152  # default compacted+padded key extent (9 tiles of 128)
B, S, E = 2, 2048, 1024
HEADS_PER_CORE = 4
D = 64
N_CORES = 8
ET = E // 128  # 8 contraction tiles for projections
QTILES = S // 512  # 4 query chunks of 512
TT = S // 128  # 16 token tiles of 128

_compiled_nc = {}


def _build_bass(kt_len=KT_LEN):
    import concourse.mybir as mybir
    import concourse.tile as tile
    from concourse import bacc

    f32 = mybir.dt.float32
    f32r = mybir.dt.float32r
    bf16 = mybir.dt.bfloat16
    Exp = mybir.ActivationFunctionType.Exp
    KT_LEN = kt_len
    KT = KT_LEN // 128
    HPC = HEADS_PER_CORE

    nc = bacc.Bacc(None, target_bir_lowering=False, debug=False)

    xT_d = nc.dram_tensor("xT", [E, S], bf16, kind="ExternalInput")
    xkvT_d = nc.dram_tensor("xkvT", [E, KT_LEN], bf16, kind="ExternalInput")
    wqT_d = nc.dram_tensor("wqT", [E, 256], bf16, kind="ExternalInput")
    wkT_d = nc.dram_tensor("wkT", [E, 256], bf16, kind="ExternalInput")
    wvT_d = nc.dram_tensor("wvT", [E, 256], bf16, kind="ExternalInput")
    woT_d = nc.dram_tensor("woT", [256, E], f32r, kind="ExternalInput")
    mb_d = nc.dram_tensor("mbias", [KT_LEN], f32, kind="ExternalInput")
    out_d = nc.dram_tensor("out", [S, E], bf16, kind="ExternalOutput")

    xT_v = xT_d[:].rearrange("(a p) t -> p a t", p=128)  # [128, 8, 2048]
    xkvT_v = xkvT_d[:].rearrange("(a p) t -> p a t", p=128)  # [128, 8, KT_LEN]
    wqT_v = wqT_d[:].rearrange("(a p) d -> p a d", p=128)  # [128, 8, 256]
    wkT_v = wkT_d[:].rearrange("(a p) d -> p a d", p=128)
    wvT_v = wvT_d[:].rearrange("(a p) d -> p a d", p=128)
    woT_v = woT_d[:].rearrange("(a p) e -> p a e", p=128)  # [128, 2, 1024]
    mb_v = mb_d[:].rearrange("(k p) -> p k", p=128)  # [128, KT]

    with tile.TileContext(nc) as tc:
        with (
            tc.tile_pool(name="singles", bufs=1) as singles,
            tc.tile_pool(name="xstream", bufs=2) as xstream,
            tc.tile_pool(name="expool", bufs=4) as expool,
            tc.tile_pool(name="small", bufs=3) as small,
            tc.tile_pool(name="outst", bufs=4) as outst,
            # PSUM: 8 banks total, statically reserved:
            #   lgp  = 2 x [128,1024] (2 banks each) -> 4 banks
            #   valsp= 3 x [65,512]   (1 bank each)  -> 3 banks
            #   opp  = 1 x [128,512]  (1 bank)       -> 1 bank
            tc.tile_pool(name="lgp", bufs=2, space="PSUM") as lgp,
            tc.tile_pool(name="valsp", bufs=3, space="PSUM") as valsp,
            tc.tile_pool(name="opp", bufs=1, space="PSUM") as opp,
        ):
            wq_sb = singles.tile([128, ET, 256], bf16)
            wk_sb = singles.tile([128, ET, 256], bf16)
            wv_sb = singles.tile([128, ET, 256], bf16)
            wo_sb = singles.tile([128, 2, E], f32r)
            # xkv split into two et-halves (separate tensors so the DMA
            # dependency of each half is exact): K accumulates its first
            # half while the second is still in flight.
            KSPL = ET // 2
            xkv_a = singles.tile([128, KSPL, KT_LEN], bf16)
            xkv_b = singles.tile([128, ET - KSPL, KT_LEN], bf16)
            mb_sb = singles.tile([128, KT], f32)
            qT_sb = singles.tile([128, 2, S // 2], bf16)
            # q-chunks 2/3 land in their own tensor (written by the filler
            # projections inside pair-0 attention) so pair-1 reads never
            # serialize against unrelated writes.
            qT2_sb = singles.tile([128, 2, S // 2], bf16)
            kT_sb = singles.tile([128, HPC, KT_LEN], bf16)
            v1_sb = singles.tile([128, KT, HPC, 65], f32r)
            valsTa = singles.tile([128, S], f32r)
            valsTb = singles.tile([128, S], f32r)

            def xkv_et(et):
                return xkv_a[:, et] if et < KSPL else xkv_b[:, et - KSPL]

            # ---- DMA prologue, in consumption order (sync queue = FIFO).
            nc.sync.dma_start(wq_sb, wqT_v)
            nc.sync.dma_start(mb_sb, mb_v)
            xq = {}
            # qc0 arrives in two et-halves so its projection starts sooner.
            xq0a = xstream.tile([128, ET // 2, 512], bf16, tag="x0", name="xq0a")
            nc.sync.dma_start(xq0a, xT_v[:, 0 : ET // 2, 0:512])
            xq0b = xstream.tile([128, ET // 2, 512], bf16, tag="x0", name="xq0b")
            nc.sync.dma_start(xq0b, xT_v[:, ET // 2 :, 0:512])
            nc.sync.dma_start(wk_sb, wkT_v)
            nc.sync.dma_start(xkv_a, xkvT_v[:, 0:KSPL])
            xq[1] = xstream.tile([128, ET, 512], bf16, tag="xs", name="xq1")
            nc.sync.dma_start(xq[1], xT_v[:, :, 512:1024])
            nc.sync.dma_start(xkv_b, xkvT_v[:, KSPL:])
            nc.sync.dma_start(wv_sb, wvT_v)
            # qc2/qc3 reuse the xstream bufs; their dma_starts wait on the
            # Q0/Q1 projections, by which time everything above has issued.
            for qc in (2, 3):
                xq[qc] = xstream.tile(
                    [128, ET, 512], bf16, tag="xs", name=f"xq{qc}"
                )
                nc.sync.dma_start(xq[qc], xT_v[:, :, qc * 512 : (qc + 1) * 512])
            nc.sync.dma_start(wo_sb, woT_v)

            # ---- constants / zero-fill (off the critical DMA+PE path)
            ones_sb = singles.tile([128, 1], f32)
            nc.vector.memset(ones_sb, 1.0)
            ones64 = singles.tile([65, 64], f32r)
            nc.scalar.copy(
                ones64[64:65, :], ones_sb[64:65, 0:1].to_broadcast([1, 64])
            )
            # unused partition halves of kT must be 0 (each head only fills
            # 64 of the 128 contraction partitions)
            zeros_sb = singles.tile([128, 1], f32)
            nc.vector.memset(zeros_sb, 0.0)
            nc.scalar.copy(
                kT_sb, zeros_sb.to_broadcast([128, HPC, KT_LEN])
            )
            nc.scalar.copy(
                v1_sb[:, :, :, 64:65],
                ones_sb.to_broadcast([128, KT, HPC, 1]),
            )

            # ---- Q projection for one 512-query chunk (full-rate version,
            # used for qc0/qc1 before attention starts).
            def q_proj_full(qc):
                psq = lgp.tile([128, 1024], f32, tag="lg", name=f"psq_{qc}")
                for et in range(ET):
                    if qc == 0:
                        rhs = (xq0a if et < ET // 2 else xq0b)[:, et % (ET // 2)]
                    else:
                        rhs = xq[qc][:, et]
                    for bl in range(2):
                        nc.tensor.matmul(
                            psq[:, bl * 512 : (bl + 1) * 512],
                            lhsT=wq_sb[:, et, bl * 128 : (bl + 1) * 128],
                            rhs=rhs,
                            start=(et == 0),
                            stop=(et == ET - 1),
                        )
                for bl in range(2):
                    nc.vector.tensor_copy(
                        qT_sb[:, bl, qc * 512 : (qc + 1) * 512],
                        psq[:, bl * 512 : (bl + 1) * 512],
                    )

            # ---- K^T projection layout: [256 d, KT_LEN] in 3 chunks x 2
            # blocks.  With <=7 groups they all stay open across the PSUM
            # banks so the accumulation runs in two et-stages: stage 0
            # (first xkv half) is sandwiched between the Q0 and Q1
            # projections while the second half is still streaming in.
            nch = (KT_LEN + 511) // 512
            base = KT_LEN // nch // 128 * 128
            KCH = []
            t0 = 0
            for ci in range(nch):
                tw = KT_LEN - t0 if ci == nch - 1 else base
                KCH.append((t0, tw))
                t0 += tw
            groups = [(bl, t0, tw) for bl in range(2) for t0, tw in KCH]

            def k_stage(homes, stage):
                ets = range(0, KSPL) if stage == 0 else range(KSPL, ET)
                for gi, (bl, t0, tw) in enumerate(groups):
                    for et in ets:
                        nc.tensor.matmul(
                            homes[gi],
                            lhsT=wk_sb[:, et, bl * 128 : (bl + 1) * 128],
                            rhs=xkv_et(et)[:, t0 : t0 + tw],
                            start=(et == 0),
                            stop=(et == ET - 1),
                        )

            q_proj_full(0)
            q_proj_full(1)
            if len(groups) <= 7:
                pskL = lgp.tile([128, 1024], f32, tag="lg", name="pskL")
                homes = []
                for gi, (bl, t0, tw) in enumerate(groups):
                    if gi == 0:
                        homes.append(pskL[:, 0:tw])
                    elif gi == 1:
                        homes.append(pskL[:, 512 : 512 + tw])
                    elif gi < 5:
                        homes.append(
                            valsp.tile([128, tw], f32, tag="vals", name=f"pskv_{gi}")
                        )
                    else:
                        homes.append(
                            opp.tile([128, tw], f32, tag="op", name=f"psko_{gi}")
                        )
                k_stage(homes, 0)
                k_stage(homes, 1)
                for gi, (bl, t0, tw) in enumerate(groups):
                    nc.vector.tensor_copy(
                        kT_sb[0:64, 2 * bl, t0 : t0 + tw], homes[gi][0:64, :]
                    )
                    nc.vector.tensor_copy(
                        kT_sb[64:128, 2 * bl + 1, t0 : t0 + tw], homes[gi][64:128, :]
                    )
            else:
                for bl in range(2):
                    for t0, tw in KCH:
                        psk = lgp.tile([128, 1024], f32, tag="lg", name=f"psk_{bl}_{t0}")
                        for et in range(ET):
                            nc.tensor.matmul(
                                psk[:, :tw],
                                lhsT=wk_sb[:, et, bl * 128 : (bl + 1) * 128],
                                rhs=xkv_et(et)[:, t0 : t0 + tw],
                                start=(et == 0),
                                stop=(et == ET - 1),
                            )
                        nc.vector.tensor_copy(
                            kT_sb[0:64, 2 * bl, t0 : t0 + tw], psk[0:64, :tw]
                        )
                        nc.vector.tensor_copy(
                            kT_sb[64:128, 2 * bl + 1, t0 : t0 + tw], psk[64:128, :tw]
                        )

            # ---- V projection: per token-tile [128 t, 256 d]
            for vt in range(KT):
                psv = lgp.tile([128, 1024], f32, tag="lg", name=f"psv_{vt}")
                for et in range(ET):
                    nc.tensor.matmul(
                        psv[:, :256],
                        lhsT=xkv_et(et)[:, vt * 128 : (vt + 1) * 128],
                        rhs=wv_sb[:, et],
                        start=(et == 0),
                        stop=(et == ET - 1),
                    )
                nc.vector.tensor_copy(
                    v1_sb[:, vt, :, 0:64],
                    psv[:, :256].rearrange("p (h d) -> p h d", h=HPC),
                )

            # ---- PE filler generators -------------------------------------
            # Pair 0 fillers: Q projection for qc2/qc3, one (qc, bl) chunk
            # split across two filler slots (4 et-steps each) in the opp bank.
            qfill_state = {}

            def emit_q_filler(i):
                qc = 2 + i // 4
                bl = (i // 2) % 2
                half = i % 2
                if half == 0:
                    qfill_state["t"] = opp.tile(
                        [128, 512], f32, tag="op", name=f"psq2_{qc}_{bl}"
                    )
                t = qfill_state["t"]
                for e4 in range(4):
                    et = half * 4 + e4
                    nc.tensor.matmul(
                        t,
                        lhsT=wq_sb[:, et, bl * 128 : (bl + 1) * 128],
                        rhs=xq[qc][:, et],
                        start=(et == 0),
                        stop=(et == ET - 1),
                    )
                if half == 1:
                    nc.vector.tensor_copy(
                        qT2_sb[:, bl, (qc - 2) * 512 : (qc - 1) * 512], t
                    )

            # Pair 1 fillers: o_proj half-tiles for pair-0 tokens (tt 0..7).
            def emit_op_filler(j):
                ttn, nt = j // 2, j % 2
                op = opp.tile([128, 512], f32, tag="op", name=f"op_{ttn}_{nt}")
                nc.tensor.matmul(
                    op,
                    lhsT=valsTa[:, ttn * 128 : (ttn + 1) * 128],
                    rhs=wo_sb[:, 0, nt * 512 : (nt + 1) * 512],
                    start=True,
                    stop=False,
                )
                nc.tensor.matmul(
                    op,
                    lhsT=valsTb[:, ttn * 128 : (ttn + 1) * 128],
                    rhs=wo_sb[:, 1, nt * 512 : (nt + 1) * 512],
                    start=False,
                    stop=True,
                )
                ot = outst.tile([128, 1024], bf16, tag="ot", name=f"ot_{ttn}_{nt}")
                nc.vector.tensor_copy(ot[:, 0:512], op)
                nc.sync.dma_start(
                    out_d[ttn * 128 : (ttn + 1) * 128, nt * 512 : (nt + 1) * 512],
                    ot[:, 0:512],
                )

            # ---- softmax-normalize one head's accumulated values ----------
            # The sumexp row (partition 64 of the AV accumulator) is spread
            # across 64 partitions on the idle GpSimd engine, keeping the
            # Tensor engine free of broadcast matmuls and the vals PSUM pool
            # free of sumexp scratch tiles.
            def emit_norm(p, h, valsA, valsB, fast=False):
                bl = h // 2
                off = (h % 2) * 64
                vT = valsTa if bl == 0 else valsTb
                uvs = []
                for X, vals in (("A", valsA), ("B", valsB)):
                    uv = small.tile([65, 512], f32r, tag="uv", name=f"uv_{p}_{h}_{X}")
                    nc.vector.tensor_copy(uv, vals)
                    uvs.append(uv)
                for xi, uv in enumerate(uvs):
                    qoff = p * 1024 + xi * 512
                    # broadcast sumexp across 64 partitions with a K=1 matmul
                    # (212ns on PE; every off-PE broadcast path measured
                    # slower end-to-end).  The tail-critical final head uses
                    # the freed filler bank instead of the vals rotation.
                    pool, tag = (opp, "op") if fast else (valsp, "vals")
                    se = pool.tile([64, 512], f32, tag=tag, name=f"se_{p}_{h}_{xi}")
                    nc.tensor.matmul(
                        se,
                        lhsT=ones64[64:65, :],
                        rhs=uv[64:65, :],
                        start=True,
                        stop=True,
                    )
                    rb = small.tile([64, 512], f32, tag="rb", name=f"rb_{p}_{h}_{xi}")
                    nc.vector.reciprocal_approx_fast(rb, se)
                    if off != 0:
                        nc.vector.tensor_mul(
                            vT[0:64, qoff : qoff + 512], uv[0:64, :], rb
                        )
                    else:
                        vn = small.tile(
                            [64, 512], f32r, tag="vn", bufs=2, name=f"vn_{p}_{h}_{xi}"
                        )
                        nc.vector.tensor_mul(vn, uv[0:64, :], rb)
                        nc.gpsimd.dma_start(vT[64:128, qoff : qoff + 512], vn)

            # ---- attention, q-pair-major ----------------------------------
            pending_norm = None
            for p in range(2):
                q0 = p * 1024
                if p == 0:
                    fillers = [(s, emit_q_filler) for s in range(8)]
                    fill_slots = {3, 7}
                else:
                    fillers = [(s, emit_op_filler) for s in range(16)]
                    fill_slots = {3, 5, 7, 8}
                fi = 0
                for h in range(HPC):
                    bl = h // 2
                    valsA = valsB = None
                    for kt in range(KT):
                        lg = lgp.tile([128, 1024], f32, tag="lg", name=f"lg_{p}_{h}_{kt}")
                        qsrc = qT_sb if p == 0 else qT2_sb
                        for xi in range(2):
                            nc.tensor.matmul(
                                lg[:, xi * 512 : (xi + 1) * 512],
                                lhsT=kT_sb[:, h, kt * 128 : (kt + 1) * 128],
                                rhs=qsrc[:, bl, xi * 512 : (xi + 1) * 512],
                                start=True,
                                stop=True,
                            )
                        ex = expool.tile([128, 1024], f32r, tag="ex", name=f"ex_{p}_{h}_{kt}")
                        nc.scalar.activation(
                            ex, lg, Exp, bias=mb_sb[:, kt : kt + 1], scale=0.125
                        )
                        # previous head's normalize lands between this head's
                        # first QKT and first AV in Tensor-engine order, so
                        # its tiny broadcast matmuls ride the exp wait.  The
                        # vals tiles are allocated after it so the pool
                        # rotation (bufs=3) frees banks in dependency order.
                        if kt == 0:
                            if pending_norm is not None:
                                emit_norm(*pending_norm)
                                pending_norm = None
                            valsA = valsp.tile(
                                [65, 512], f32, tag="vals", name=f"vals_{p}_{h}_A"
                            )
                            valsB = valsp.tile(
                                [65, 512], f32, tag="vals", name=f"vals_{p}_{h}_B"
                            )
                        nc.tensor.matmul(
                            valsA,
                            lhsT=v1_sb[:, kt, h],
                            rhs=ex[:, 0:512],
                            start=(kt == 0),
                            stop=(kt == KT - 1),
                        )
                        nc.tensor.matmul(
                            valsB,
                            lhsT=v1_sb[:, kt, h],
                            rhs=ex[:, 512:1024],
                            start=(kt == 0),
                            stop=(kt == KT - 1),
                        )
                        if fi < len(fillers) and (h * KT + kt) % KT in fill_slots:
                            fillers[fi][1](fillers[fi][0])
                            fi += 1
                    pending_norm = (p, h, valsA, valsB)
                # drain any leftover fillers (KT-size changes etc.)
                while fi < len(fillers):
                    fillers[fi][1](fillers[fi][0])
                    fi += 1

            emit_norm(*pending_norm, fast=True)

            # ---- o_proj for pair-1 tokens (tt 8..15): nothing left to
            # overlap with, so spread 16 half-tiles over all 8 freed PSUM
            # banks and drain with both the Scalar and Vector engines.
            def op_homes():
                while True:
                    lga = lgp.tile([128, 1024], f32, tag="lg", name="opfA")
                    lgb = lgp.tile([128, 1024], f32, tag="lg", name="opfB")
                    yield lga[:, 0:512]
                    yield lga[:, 512:1024]
                    yield lgb[:, 0:512]
                    yield lgb[:, 512:1024]
                    for k in range(3):
                        yield valsp.tile([128, 512], f32, tag="vals", name=f"opfv{k}")
                    yield opp.tile([128, 512], f32, tag="op", name="opfo")

            homegen = op_homes()
            for ttn in range(TT // 2, TT):
                ot = outst.tile([128, 1024], bf16, tag="ot", name=f"otf_{ttn}")
                for ntn in range(2):
                    op = next(homegen)
                    nc.tensor.matmul(
                        op,
                        lhsT=valsTa[:, ttn * 128 : (ttn + 1) * 128],
                        rhs=wo_sb[:, 0, ntn * 512 : (ntn + 1) * 512],
                        start=True,
                        stop=False,
                    )
                    nc.tensor.matmul(
                        op,
                        lhsT=valsTb[:, ttn * 128 : (ttn + 1) * 128],
                        rhs=wo_sb[:, 1, ntn * 512 : (ntn + 1) * 512],
                        start=False,
                        stop=True,
                    )
                    # halves drain on different engines; one fat store per tile
                    if ntn == 0:
                        nc.scalar.copy(ot[:, 0:512], op)
                    else:
                        nc.vector.tensor_copy(ot[:, 512:1024], op)
                nc.sync.dma_start(out_d[ttn * 128 : (ttn + 1) * 128, :], ot)

    nc.compile()
    return nc


def _get_nc(kt_len=KT_LEN):
    if kt_len not in _compiled_nc:
        _compiled_nc[kt_len] = _build_bass(kt_len)
    return _compiled_nc[kt_len]


def pick_kt_len(src_padding_mask):
    """Smallest supported compacted key extent covering every batch's kept
    tokens (KT_LEN default covers it with ~5 sigma of slack for random
    masks; anything larger falls back to a wider, slower build)."""
    need = int(np.max(np.sum(np.asarray(src_padding_mask), axis=1)))
    need = max(need, 256)
    need = (need + 127) // 128 * 128
    return KT_LEN if need <= KT_LEN else need


def make_in_maps(x, src_padding_mask, w_qkv, w_o, kt_len=None):
    """Shard the full inputs into the 8 per-core input maps."""
    import ml_dtypes

    bf16 = ml_dtypes.bfloat16
    if kt_len is None:
        kt_len = pick_kt_len(src_padding_mask)
    x = np.asarray(x, dtype=np.float32)
    mask = np.asarray(src_padding_mask)
    w_qkv = np.asarray(w_qkv, dtype=np.float32)
    w_o = np.asarray(w_o, dtype=np.float32)

    # w_qkv rows are per-head interleaved: head h -> rows [192h, 192h+192),
    # split 64/64/64 into q/k/v.
    wr = w_qkv.reshape(16, 3, D, E)  # [head, qkv, d, e]

    in_maps = []
    per_batch = {}
    for b in range(B):
        xb = x[b]  # [S, E]
        xT = np.ascontiguousarray(xb.T)
        idx = np.nonzero(mask[b])[0]
        nk = len(idx)
        assert nk <= kt_len, f"kept keys {nk} exceed kt_len {kt_len}"
        xkvT = np.zeros((E, kt_len), np.float32)
        xkvT[:, :nk] = xb[idx].T
        mb = np.full((kt_len,), -30000.0, np.float32)
        mb[:nk] = 0.0
        per_batch[b] = (xT, xkvT, mb)

    for c in range(N_CORES):
        b, g = divmod(c, N_CORES // B)
        xT, xkvT, mb = per_batch[b]
        heads = slice(g * HEADS_PER_CORE, (g + 1) * HEADS_PER_CORE)
        wq = wr[heads, 0].reshape(256, E)  # [4*64, E]
        wk = wr[heads, 1].reshape(256, E)
        wv = wr[heads, 2].reshape(256, E)
        in_maps.append(
            {
                "xT": xT.astype(bf16),
                "xkvT": xkvT.astype(bf16),
                "wqT": np.ascontiguousarray(wq.T).astype(bf16),
                "wkT": np.ascontiguousarray(wk.T).astype(bf16),
                "wvT": np.ascontiguousarray(wv.T).astype(bf16),
                "woT": np.ascontiguousarray(
                    w_o[:, g * 256 : (g + 1) * 256]
                    .reshape(E, 2, 2, D)[:, :, ::-1, :]
                    .reshape(E, 256)
                    .T
                ),
                "mbias": mb,
            }
        )
    return in_maps


def combine_outputs(outs):
    """Sum the 4 per-head-group partials for each batch."""
    full = np.zeros((B, S, E), np.float32)
    for c in range(N_CORES):
        full[c // (N_CORES // B)] += np.asarray(outs[c]).astype(np.float32)
    return full


def kernel(x, src_padding_mask, w_qkv, w_o, _trace=False):
    from concourse.bass_utils import run_bass_kernel_spmd

    kt_len = pick_kt_len(src_padding_mask)
    nc = _get_nc(kt_len)
    in_maps = make_in_maps(x, src_padding_mask, w_qkv, w_o, kt_len)
    kwargs = {}
    if _trace:
        kwargs = dict(trace=True, trace_cores=list(range(N_CORES)))
    res = run_bass_kernel_spmd(nc, in_maps, core_ids=list(range(N_CORES)), **kwargs)
    out = combine_outputs([r["out"] for r in res.results])
    if _trace:
        kernel._last_result = res
    return out
